# revision 20
# baseline (speedup 1.0000x reference)
"""Batch-softmax dot-product attention on 8 trn2 NeuronCores.

reference:  S = einsum('bqd,bkd->bqk', Q, K) / sqrt(D)
            A = softmax(S, axis=0)            # over the BATCH dim!
            out = einsum('bqk,bkd->bqd', A, V)

Sharding: split the QUERY dim across the 8 cores (256 queries each).
The softmax couples (q, k) positions across batches only, so with all
16 batches resident per core the kernel is embarrassingly parallel —
no collectives.

Per-core dataflow (all matmuls bf16 in / f32 PSUM out):
  mm1   S^T[b] (ktile 128 x q 256) = K^T[b] . Q[b]   contract d=64;
        even b on SBUF partitions 0-63, odd b on 64-127 (row-tiled PE)
  exp   ACT: P = exp(S^T / 8)  f32 PSUM -> bf16 SBUF (max-free softmax:
        scores are ~N(0,1), |s|<9, exp never overflows fp32)
  zsum  Z = sum_b P[b] via PE identity-matmul accumulation (exact f32)
  recip R = 1/Z (DVE reciprocal_approx_accurate) -> bf16
  mul   attn = P * R (DVE, broadcast R over b)
  mm2   outT pair [128(2 x d64), q256] += V[b]^T . attn[b], accumulated
        over all 16 ktiles in PSUM; odd b targets PSUM partitions
        64-127 via PE column tiling.

PSUM per partition (16KB): slotA 4KB + slotB 2KB (S^T staging) +
Z 2KB + 8 outT pair accumulators 8KB.
"""

import numpy as np
import ml_dtypes

import concourse.bass as bass
import concourse.bacc as bacc
import concourse.tile as tile_mod
from concourse import mybir
from concourse.bass_utils import run_bass_kernel_spmd

B, N, D = 16, 2048, 64
NCORES = 8
QL = N // NCORES           # 256 queries per core
KT = 128                   # keys per kt tile
TK = 2                     # kt tiles per group
NG = N // (KT * TK)        # 8 groups
BP = B // 2                # 8 batch pairs
BF = mybir.dt.bfloat16
F32 = mybir.dt.float32
SCALE = 1.0 / np.sqrt(D)

bf16 = ml_dtypes.bfloat16


def build_program(stage="full", ngroups=NG, skip_odd=False):
    # stage: "dma" | "mm1" | "softmax" | "full" — for HW bisection.
    # Bacc (not raw Bass): its compile() pass moves extra matmul waits onto
    # ldweights / event-semaphores, which walrus codegen requires.
    nc = bacc.Bacc(trn_type="TRN2")

    # (bo d) packing: partition p = 64*(b%2) + d so odd batches live on
    # partitions 64-127 (PE row tiling T8 for mm1).
    qT = nc.dram_tensor("qT", [B, D, QL], BF, kind="ExternalInput")
    kT = nc.dram_tensor("kT", [B, D, N], BF, kind="ExternalInput")
    v3 = nc.dram_tensor("v3", [N, B, D], BF, kind="ExternalInput")
    out = nc.dram_tensor("out", [BP, 2, D, QL], F32, kind="ExternalOutput")

    ident = nc.inline_tensor(np.eye(128, dtype=bf16), name="ident")

    qT_r = qT.rearrange("(bp bo) d q -> (bo d) bp q", bo=2)
    kT_r = kT.rearrange("(bp bo) d n -> (bo d) bp n", bo=2)
    out_r = out.rearrange("bp bo d q -> (bo d) bp q")

    with tile_mod.TileContext(nc) as tc:
        with (
            tc.tile_pool(name="singles", bufs=1) as singles,
            tc.tile_pool(name="kt", bufs=2) as kt_pool,
            tc.tile_pool(name="v", bufs=2) as v_pool,
            tc.tile_pool(name="p", bufs=2) as p_pool,
            tc.tile_pool(name="attn", bufs=2) as a_pool,
            tc.tile_pool(name="r", bufs=2) as r_pool,
            # PSUM allocation order fixes bank placement.
            tc.tile_pool(name="psA", bufs=1, space="PSUM") as psA_pool,
            tc.tile_pool(name="psB", bufs=1, space="PSUM") as psB_pool,
            tc.tile_pool(name="psZ", bufs=1, space="PSUM") as psZ_pool,
            tc.tile_pool(name="psO", bufs=1, space="PSUM") as psO_pool,
        ):
            qt_sb = singles.tile([128, BP, QL], BF)
            nc.sync.dma_start(out=qt_sb, in_=qT_r)
            id_sb = singles.tile([128, 128], BF)
            nc.sync.dma_start(out=id_sb, in_=ident[:, :])

            # Persistent PSUM accumulators, one full bank each: tile j
            # holds batch pairs 2j (cols 0-255) and 2j+1 (cols 256-511);
            # within a pair, even b on partitions 0-63, odd b on 64-127.
            outacc = [psO_pool.tile([128, 2 * QL], F32, tag=f"o{i}", name=f"outacc{i}") for i in range(BP // 2)]

            # mm1/exp slot schedule per kt tile. HW constraint (probed):
            # row-tiled matmuls that ALTERNATE tile_position (T0/T8) within
            # one PSUM bank accumulation group crash the device — each bank
            # must be filled by uniform-parity batches. Slot A = 2 banks
            # (4 tiles), slot B = 1 bank (2 tiles).
            SLOT_SCHED = [
                ("A", [0, 2, 1, 3]),
                ("B", [4, 6]),
                ("A", [5, 7, 8, 10]),
                ("B", [9, 11]),
                ("A", [12, 14, 13, 15]),
            ]

            for g in range(ngroups):
                kt_sb = kt_pool.tile([128, BP, TK * KT], BF, tag="kt")
                nc.sync.dma_start(
                    out=kt_sb, in_=kT_r[:, :, g * TK * KT:(g + 1) * TK * KT]
                )
                v_sb = v_pool.tile([128, B, TK, D], BF, tag="v")
                nc.sync.dma_start(
                    out=v_sb,
                    in_=v3[g * TK * KT:(g + 1) * TK * KT].rearrange(
                        "(t p) b d -> p b t d", t=TK
                    ),
                )

                P = p_pool.tile([128, B, TK * QL], BF, tag="p")
                A = a_pool.tile([128, B, TK * QL], BF, tag="a")

                if stage != "full" and g == 0:
                    for i in range(BP // 2):
                        nc.vector.memset(outacc[i][:, :], 0.0)
                if stage == "dma":
                    nc.vector.tensor_copy(out=P[:, 0, 0:TK * KT], in_=kt_sb[:, 0, :])
                    nc.vector.tensor_copy(out=A[:, 0, 0:TK * D],
                                          in_=v_sb[:, 0, :, :].rearrange("p t d -> p (t d)"))
                    continue

                BS = TK * QL    # element stride between batches in P
                for t in range(TK):
                    for slot, bl in SLOT_SCHED:
                        nb = len(bl)
                        if slot == "A":
                            s_ps = psA_pool.tile([128, 4 * QL], F32, tag="sa")
                        else:
                            s_ps = psB_pool.tile([128, 2 * QL], F32, tag="sb")
                        for i, b in enumerate(bl):
                            bo, bp = b % 2, b // 2
                            if skip_odd and bo == 1:
                                continue
                            # start=True lazy-zeroes the whole 2KB PSUM bank,
                            # so exactly one start/stop pair per bank (two
                            # 256-col slices share each bank).
                            nc.tensor.matmul(
                                out=s_ps[:, i * QL:(i + 1) * QL],
                                lhsT=kt_sb[bo * 64:(bo + 1) * 64, bp,
                                           t * KT:(t + 1) * KT],
                                rhs=qt_sb[bo * 64:(bo + 1) * 64, bp, :],
                                start=(i % 2 == 0), stop=(i % 2 == 1),
                            )
                        # exp writes P[b] slices in PSUM stream order; the
                        # b-permutation becomes a strided 3/4D output AP.
                        p_ap = P[:, :, :]
                        off = p_ap.offset + bl[0] * BS + t * QL
                        if nb == 4:
                            free = [[(bl[2] - bl[0]) * BS, 2],
                                    [(bl[1] - bl[0]) * BS, 2], [1, QL]]
                            in_ap = s_ps[:, :].rearrange(
                                "p (o i q) -> p o i q", o=2, i=2)
                        else:
                            free = [[(bl[1] - bl[0]) * BS, 2], [1, QL]]
                            in_ap = s_ps[:, :].rearrange(
                                "p (i q) -> p i q", i=2)
                        out_ap = bass.AP(tensor=p_ap.tensor, offset=off,
                                         ap=[p_ap.ap[0]] + free)
                        nc.scalar.activation(
                            out=out_ap,
                            in_=in_ap,
                            func=mybir.ActivationFunctionType.Exp,
                            scale=SCALE,
                        )

                if stage == "mm1":
                    nc.vector.tensor_copy(out=A[:, 0, :], in_=P[:, 0, :])
                    continue

                # Z = sum_b P[b] : identity-matmul accumulation in PSUM.
                Zp = psZ_pool.tile([128, TK * QL], F32, tag="z")
                for b in range(B):
                    nc.tensor.matmul(
                        out=Zp[:, :],
                        lhsT=id_sb[:, :],
                        rhs=P[:, b, :],
                        start=(b == 0), stop=(b == B - 1),
                    )

                Rf = r_pool.tile([128, TK * QL], F32, tag="rf")
                Rs = r_pool.tile([128, TK * QL], F32, tag="rs")
                Rb = r_pool.tile([128, TK * QL], BF, tag="rb")
                nc.vector.reciprocal_approx_accurate(out=Rf[:, :], in_=Zp[:, :], scratch=Rs[:, :])
                nc.vector.tensor_copy(out=Rb[:, :], in_=Rf[:, :])

                # attn = P * R, R broadcast across the batch dim.
                rb_ap = Rb[:, :]
                rb_bcast = bass.AP(
                    tensor=rb_ap.tensor,
                    offset=rb_ap.offset,
                    ap=[rb_ap.ap[0], [0, B], rb_ap.ap[1]],
                )
                nc.vector.tensor_mul(out=A[:, :, :], in0=P[:, :, :], in1=rb_bcast)

                if stage == "softmax":
                    continue

                for b in range(B):
                    bo, bp = b % 2, b // 2
                    for t in range(TK):
                        nc.tensor.matmul(
                            out=outacc[bp // 2][bo * 64:(bo + 1) * 64,
                                                (bp % 2) * QL:(bp % 2 + 1) * QL],
                            lhsT=v_sb[:, b, t, :],
                            rhs=A[:, b, t * QL:(t + 1) * QL],
                            # zero regions are per-partition banks: the two
                            # col-slices (bp%2) share one; partition halves
                            # (bo) are independent.
                            start=(g == 0 and t == 0 and bp % 2 == 0),
                            stop=(g == NG - 1 and t == TK - 1 and bp % 2 == 1),
                            # CoreSim's group tracker is partition-base
                            # blind; data semantics verified separately.
                            skip_group_check=True,
                        )

            out_sb = singles.tile([128, BP, QL], F32)
            for i in range(BP // 2):
                nc.vector.tensor_copy(
                    out=out_sb[:, 2 * i:2 * i + 2, :],
                    in_=outacc[i][:, :].rearrange("p (j q) -> p j q", j=2),
                )
            nc.sync.dma_start(out=out_r, in_=out_sb)

    nc.finalize()   # Bacc.compile(): reg alloc + wait legalization
    return nc


_NC_CACHE = None


def _get_program():
    global _NC_CACHE
    if _NC_CACHE is None:
        _NC_CACHE = build_program()
    return _NC_CACHE


def make_in_maps(queries, keys, values):
    """Host-side staging: transpose/cast/shard. Returns per-core input maps."""
    kT = np.ascontiguousarray(keys.transpose(0, 2, 1)).astype(bf16)      # [B, D, N]
    v3 = np.ascontiguousarray(values.transpose(1, 0, 2)).astype(bf16)    # [N, B, D]
    in_maps = []
    for c in range(NCORES):
        qs = queries[:, c * QL:(c + 1) * QL, :]
        qT = np.ascontiguousarray(qs.transpose(0, 2, 1)).astype(bf16)    # [B, D, QL]
        in_maps.append({"qT": qT, "kT": kT, "v3": v3})
    return in_maps


def assemble_output(results):
    """[BP, 2, D, QL] per core -> [B, N, D] full output."""
    out = np.empty((B, N, D), dtype=np.float32)
    for c, res in enumerate(results):
        oc = res["out"]                      # [8, 2, 64, 256]
        oc = oc.reshape(B, D, QL)            # b = bp*2 + bo
        out[:, c * QL:(c + 1) * QL, :] = oc.transpose(0, 2, 1)
    return out


def kernel(queries, keys, values):
    nc = _get_program()
    in_maps = make_in_maps(queries, keys, values)
    res = run_bass_kernel_spmd(nc, in_maps, core_ids=list(range(NCORES)))
    return assemble_output(res.results)


if __name__ == "__main__":
    rng = np.random.default_rng(0)
    q = rng.standard_normal((B, N, D), dtype=np.float32)
    k = rng.standard_normal((B, N, D), dtype=np.float32)
    v = rng.standard_normal((B, N, D), dtype=np.float32)
    o = kernel(queries=q, keys=k, values=v)
    print("kernel output", o.shape, o.dtype)


# revision 21
# speedup vs baseline: 1.0001x; 1.0001x over previous
"""Batch-softmax dot-product attention on 8 trn2 NeuronCores.

reference:  S = einsum('bqd,bkd->bqk', Q, K) / sqrt(D)
            A = softmax(S, axis=0)            # over the BATCH dim!
            out = einsum('bqk,bkd->bqd', A, V)

Sharding: split the QUERY dim across the 8 cores (256 queries each).
The softmax couples (q, k) positions across batches only, so with all
16 batches resident per core the kernel is embarrassingly parallel —
no collectives.

Host staging: every DRAM tensor is laid out as the exact SBUF
partition-image the kernel wants, so each DMA is one contiguous
multi-KB chunk per partition (descriptor-count, not bandwidth, was
the bottleneck with natural layouts). Partition packing for Q/K/out:
p = 64*(b%2) + d — odd batches live on partitions 64-127, which rows
both the mm1 row tiling (T0/T8) and the mm2 column tiling need.

Per-core dataflow (all matmuls bf16 in / f32 PSUM out):
  mm1   S^T[b] (ktile 128 x q 256) = K^T[b] . Q[b]   contract d=64,
        row-tiled: even b uses PE tile T0, odd b T8. HW constraint
        (probed): T0/T8 must not alternate within one PSUM bank
        accumulation group, so the slot schedule fills each bank with
        uniform-parity batches.
  exp   ACT: P = exp(S^T / 8)  f32 PSUM -> bf16 SBUF (max-free
        softmax: scores are ~N(0,1), exp never overflows)
  zsum  Z = sum_b P[b] via PE identity-matmul accumulation (exact f32)
  recip R = 1/Z (DVE reciprocal_approx_accurate) -> bf16
  mul   attn = P * R (DVE, broadcast R over b)
  mm2   outT pair [128(2 x d64), q256] += V[b]^T . attn[b], accumulated
        over all 16 ktiles in PSUM; odd b targets PSUM partitions
        64-127 via PE column tiling.

PSUM per partition (16KB): slotA 4KB + slotB 2KB (S^T staging) +
Z 2KB + 4 outT pair-accumulator banks 8KB.
"""

import numpy as np
import ml_dtypes

import concourse.bass as bass
import concourse.bacc as bacc
import concourse.tile as tile_mod
from concourse import mybir
from concourse.bass_utils import run_bass_kernel_spmd

B, N, D = 16, 2048, 64
NCORES = 8
QL = N // NCORES           # 256 queries per core
KT = 128                   # keys per kt tile
TK = 2                     # kt tiles per group
NG = N // (KT * TK)        # 8 groups
BP = B // 2                # 8 batch pairs
BF = mybir.dt.bfloat16
F32 = mybir.dt.float32
SCALE = 1.0 / np.sqrt(D)

bf16 = ml_dtypes.bfloat16


def build_program(stage="full"):
    # Bacc (not raw Bass): its compile() pass moves extra matmul waits onto
    # ldweights / event-semaphores, which walrus codegen requires.
    nc = bacc.Bacc(trn_type="TRN2")

    # SBUF partition-images (see module docstring).
    qH = nc.dram_tensor("qH", [128, BP, QL], BF, kind="ExternalInput")
    kH = nc.dram_tensor("kH", [128, BP, N], BF, kind="ExternalInput")
    vH = nc.dram_tensor("vH", [NG, 128, TK, B, D], BF, kind="ExternalInput")
    outH = nc.dram_tensor("outH", [128, BP, QL], F32, kind="ExternalOutput")

    ident = nc.inline_tensor(np.eye(128, dtype=bf16), name="ident")

    with tile_mod.TileContext(nc) as tc:
        with (
            tc.tile_pool(name="singles", bufs=1) as singles,
            tc.tile_pool(name="v", bufs=2) as v_pool,
            tc.tile_pool(name="p", bufs=2) as p_pool,
            tc.tile_pool(name="attn", bufs=2) as a_pool,
            tc.tile_pool(name="r", bufs=2) as r_pool,
            # PSUM allocation order fixes bank placement.
            tc.tile_pool(name="psA", bufs=1, space="PSUM") as psA_pool,
            tc.tile_pool(name="psB", bufs=1, space="PSUM") as psB_pool,
            tc.tile_pool(name="psZ", bufs=1, space="PSUM") as psZ_pool,
            tc.tile_pool(name="psO", bufs=1, space="PSUM") as psO_pool,
        ):
            qt_sb = singles.tile([128, BP, QL], BF)
            nc.sync.dma_start(out=qt_sb, in_=qH[:, :, :])
            kt_all = singles.tile([128, BP, N], BF)
            nc.sync.dma_start(out=kt_all, in_=kH[:, :, :])
            id_sb = singles.tile([128, 128], BF)
            nc.sync.dma_start(out=id_sb, in_=ident[:, :])

            # Persistent PSUM accumulators, one full bank each: tile j
            # holds batch pairs 2j (cols 0-255) and 2j+1 (cols 256-511);
            # within a pair, even b on partitions 0-63, odd b on 64-127.
            outacc = [psO_pool.tile([128, 2 * QL], F32, tag=f"o{i}", name=f"outacc{i}")
                      for i in range(BP // 2)]

            # mm1/exp slot schedule per kt tile: each PSUM bank is filled
            # by a uniform-parity batch pair (HW row-tiling constraint).
            SLOT_SCHED = [
                ("A", [0, 2, 1, 3]),
                ("B", [4, 6]),
                ("A", [5, 7, 8, 10]),
                ("B", [9, 11]),
                ("A", [12, 14, 13, 15]),
            ]

            for g in range(NG):
                v_sb = v_pool.tile([128, TK, B, D], BF, tag="v")
                nc.sync.dma_start(out=v_sb, in_=vH[g])

                P = p_pool.tile([128, B, TK * QL], BF, tag="p")
                A = a_pool.tile([128, B, TK * QL], BF, tag="a")

                if stage != "full" and g == 0:
                    for i in range(BP // 2):
                        nc.vector.memset(outacc[i][:, :], 0.0)
                if stage == "dma":
                    nc.vector.tensor_copy(out=P[:, 0, 0:TK * KT],
                                          in_=kt_all[:, 0, 0:TK * KT])
                    nc.vector.tensor_copy(
                        out=A[:, 0, 0:TK * D],
                        in_=v_sb[:, :, 0, :].rearrange("p t d -> p (t d)"))
                    continue

                BS = TK * QL    # element stride between batches in P
                for t in range(TK):
                    for slot, bl in SLOT_SCHED:
                        nb = len(bl)
                        if slot == "A":
                            s_ps = psA_pool.tile([128, 4 * QL], F32, tag="sa")
                        else:
                            s_ps = psB_pool.tile([128, 2 * QL], F32, tag="sb")
                        for i, b in enumerate(bl):
                            bo, bp = b % 2, b // 2
                            # start=True lazy-zeroes the whole 2KB PSUM bank,
                            # so exactly one start/stop pair per bank (two
                            # 256-col slices share each bank).
                            nc.tensor.matmul(
                                out=s_ps[:, i * QL:(i + 1) * QL],
                                lhsT=kt_all[bo * 64:(bo + 1) * 64, bp,
                                            (g * TK + t) * KT:(g * TK + t + 1) * KT],
                                rhs=qt_sb[bo * 64:(bo + 1) * 64, bp, :],
                                start=(i % 2 == 0), stop=(i % 2 == 1),
                            )
                        # exp writes P[b] slices in PSUM stream order; the
                        # b-permutation becomes a strided 3/4D output AP.
                        p_ap = P[:, :, :]
                        off = p_ap.offset + bl[0] * BS + t * QL
                        if nb == 4:
                            free = [[(bl[2] - bl[0]) * BS, 2],
                                    [(bl[1] - bl[0]) * BS, 2], [1, QL]]
                            in_ap = s_ps[:, :].rearrange(
                                "p (o i q) -> p o i q", o=2, i=2)
                        else:
                            free = [[(bl[1] - bl[0]) * BS, 2], [1, QL]]
                            in_ap = s_ps[:, :].rearrange(
                                "p (i q) -> p i q", i=2)
                        out_ap = bass.AP(tensor=p_ap.tensor, offset=off,
                                         ap=[p_ap.ap[0]] + free)
                        nc.scalar.activation(
                            out=out_ap,
                            in_=in_ap,
                            func=mybir.ActivationFunctionType.Exp,
                            scale=SCALE,
                        )

                if stage == "mm1":
                    nc.vector.tensor_copy(out=A[:, 0, :], in_=P[:, 0, :])
                    continue

                # Z = sum_b P[b] : identity-matmul accumulation in PSUM.
                Zp = psZ_pool.tile([128, TK * QL], F32, tag="z")
                for b in range(B):
                    nc.tensor.matmul(
                        out=Zp[:, :],
                        lhsT=id_sb[:, :],
                        rhs=P[:, b, :],
                        start=(b == 0), stop=(b == B - 1),
                    )

                Rf = r_pool.tile([128, TK * QL], F32, tag="rf")
                Rs = r_pool.tile([128, TK * QL], F32, tag="rs")
                Rb = r_pool.tile([128, TK * QL], BF, tag="rb")
                nc.vector.reciprocal_approx_accurate(out=Rf[:, :], in_=Zp[:, :],
                                                     scratch=Rs[:, :])
                nc.vector.tensor_copy(out=Rb[:, :], in_=Rf[:, :])

                # attn = P * R, R broadcast across the batch dim.
                rb_ap = Rb[:, :]
                rb_bcast = bass.AP(
                    tensor=rb_ap.tensor,
                    offset=rb_ap.offset,
                    ap=[rb_ap.ap[0], [0, B], rb_ap.ap[1]],
                )
                nc.vector.tensor_mul(out=A[:, :, :], in0=P[:, :, :], in1=rb_bcast)

                if stage == "softmax":
                    continue

                for b in range(B):
                    bo, bp = b % 2, b // 2
                    for t in range(TK):
                        nc.tensor.matmul(
                            out=outacc[bp // 2][bo * 64:(bo + 1) * 64,
                                                (bp % 2) * QL:(bp % 2 + 1) * QL],
                            lhsT=v_sb[:, t, b, :],
                            rhs=A[:, b, t * QL:(t + 1) * QL],
                            # zero regions are per-partition banks: the two
                            # col-slices (bp%2) share one; partition halves
                            # (bo) are independent.
                            start=(g == 0 and t == 0 and bp % 2 == 0),
                            stop=(g == NG - 1 and t == TK - 1 and bp % 2 == 1),
                            # CoreSim's group tracker is partition-base
                            # blind; data semantics verified separately.
                            skip_group_check=True,
                        )

            out_sb = singles.tile([128, BP, QL], F32)
            for i in range(BP // 2):
                nc.vector.tensor_copy(
                    out=out_sb[:, 2 * i:2 * i + 2, :],
                    in_=outacc[i][:, :].rearrange("p (j q) -> p j q", j=2),
                )
            nc.sync.dma_start(out=outH[:, :, :], in_=out_sb)

    nc.finalize()   # Bacc.compile(): reg alloc + wait legalization
    return nc


_NC_CACHE = None


def _get_program():
    global _NC_CACHE
    if _NC_CACHE is None:
        _NC_CACHE = build_program()
    return _NC_CACHE


def make_in_maps(queries, keys, values):
    """Host-side staging into SBUF partition-images (bf16)."""
    # kH[64*bo + d, bp, k] = K[2*bp + bo, k, d]
    kH = np.ascontiguousarray(
        keys.reshape(BP, 2, N, D).transpose(1, 3, 0, 2)
    ).reshape(128, BP, N).astype(bf16)
    # vH[g, p, t, b, d] = V[b, g*256 + t*128 + p, d]
    vH = np.ascontiguousarray(
        values.reshape(B, NG, TK, KT, D).transpose(1, 3, 2, 0, 4)
    ).astype(bf16)
    in_maps = []
    for c in range(NCORES):
        qs = queries[:, c * QL:(c + 1) * QL, :]          # [B, QL, D]
        qH = np.ascontiguousarray(
            qs.reshape(BP, 2, QL, D).transpose(1, 3, 0, 2)
        ).reshape(128, BP, QL).astype(bf16)
        in_maps.append({"qH": qH, "kH": kH, "vH": vH})
    return in_maps


def assemble_output(results):
    """outH [128, BP, QL] per core -> [B, N, D] full output."""
    out = np.empty((B, N, D), dtype=np.float32)
    for c, res in enumerate(results):
        oc = res["outH"].reshape(2, D, BP, QL).transpose(2, 0, 3, 1)  # [bp, bo, q, d]
        out[:, c * QL:(c + 1) * QL, :] = oc.reshape(B, QL, D)
    return out


def kernel(queries, keys, values):
    nc = _get_program()
    in_maps = make_in_maps(queries, keys, values)
    res = run_bass_kernel_spmd(nc, in_maps, core_ids=list(range(NCORES)))
    return assemble_output(res.results)


if __name__ == "__main__":
    rng = np.random.default_rng(0)
    q = rng.standard_normal((B, N, D), dtype=np.float32)
    k = rng.standard_normal((B, N, D), dtype=np.float32)
    v = rng.standard_normal((B, N, D), dtype=np.float32)
    o = kernel(queries=q, keys=k, values=v)
    print("kernel output", o.shape, o.dtype)


# revision 23
# speedup vs baseline: 52.2450x; 52.2419x over previous
"""Batch-softmax dot-product attention on 8 trn2 NeuronCores.

reference:  S = einsum('bqd,bkd->bqk', Q, K) / sqrt(D)
            A = softmax(S, axis=0)            # over the BATCH dim!
            out = einsum('bqk,bkd->bqd', A, V)

Sharding: split the QUERY dim across the 8 cores (256 queries each).
The softmax couples (q, k) positions across batches only, so with all
16 batches resident per core the kernel is embarrassingly parallel —
no collectives.

Host staging: every DRAM tensor is laid out as the exact SBUF
partition-image the kernel wants, so each DMA is one contiguous
multi-KB chunk per partition (descriptor count, not bandwidth, was the
bottleneck with natural layouts). Partition packing for Q/K/out:
p = 64*(b%2) + d — odd batches live on partitions 64-127, which both
the mm1 row tiling (T0/T8) and the mm2 column tiling need.

Per-core dataflow (all matmuls bf16 in / f32 PSUM out):
  mm1   S^T[b] (ktile 128 x q 256) = K^T[b] . Q[b]   contract d=64,
        row-tiled: even b uses PE tile T0, odd b T8. HW constraint
        (probed): T0/T8 must not alternate within one PSUM bank
        accumulation group, so the slot schedule fills each bank with
        uniform-parity batches.
  exp   ACT: P = exp(S^T / 8)  f32 PSUM -> bf16 SBUF (max-free
        softmax: scores are ~N(0,1), exp never overflows)
  zsum  Z = sum_b P[b] via PE identity-matmul accumulation (exact f32)
  recip R = 1/Z (DVE reciprocal_approx_accurate) -> bf16
  mul   attn = P * R (DVE, broadcast R over b)
  mm2   outT pair [128(2 x d64), q256] += V[b]^T . attn[b], accumulated
        over all 16 ktiles in PSUM; odd b targets PSUM partitions
        64-127 via PE column tiling.

Software pipeline (PE is in-order; without it the PE queue stalls on
the DVE recip+mul chain every group): iteration g emits
  mm1+exp(g) | zsum(g-1) recip/mul(g-1) | mm2(g-2)
so the PE always has mm1(g) ready while DVE normalizes g-1.

PSUM per partition (16KB): slotA 4KB + slotB 2KB (S^T staging) +
Z 2KB + 4 outT pair-accumulator banks 8KB.
"""

import numpy as np
import ml_dtypes

import concourse.bass as bass
import concourse.bacc as bacc
import concourse.tile as tile_mod
from concourse import mybir
from concourse.bass_utils import run_bass_kernel_spmd

B, N, D = 16, 2048, 64
NCORES = 8
QL = N // NCORES           # 256 queries per core
KT = 128                   # keys per kt tile
TK = 2                     # kt tiles per group
NG = N // (KT * TK)        # 8 groups
BP = B // 2                # 8 batch pairs
GK = TK * KT               # 256 keys per group
BF = mybir.dt.bfloat16
F32 = mybir.dt.float32
SCALE = 1.0 / np.sqrt(D)

bf16 = ml_dtypes.bfloat16

# mm1/exp slot schedule per kt tile: each PSUM bank is filled by a
# uniform-parity batch pair (HW row-tiling constraint).
SLOT_SCHED = [
    ("A", [0, 2, 1, 3]),
    ("B", [4, 6]),
    ("A", [5, 7, 8, 10]),
    ("B", [9, 11]),
    ("A", [12, 14, 13, 15]),
]


def build_program(repeat=1):
    # Bacc (not raw Bass): its compile() pass moves extra matmul waits onto
    # ldweights / event-semaphores, which walrus codegen requires.
    nc = bacc.Bacc(trn_type="TRN2")

    # SBUF partition-images (see module docstring).
    qH = nc.dram_tensor("qH", [128, BP, QL], BF, kind="ExternalInput")
    kH = nc.dram_tensor("kH", [NG, 128, BP, GK], BF, kind="ExternalInput")
    vH = nc.dram_tensor("vH", [NG, 128, TK, B, D], BF, kind="ExternalInput")
    outH = nc.dram_tensor("outH", [128, BP, QL], F32, kind="ExternalOutput")

    ident = nc.inline_tensor(np.eye(128, dtype=bf16), name="ident")

    with tile_mod.TileContext(nc) as tc:
        with (
            tc.tile_pool(name="singles", bufs=1) as singles,
            tc.tile_pool(name="kt", bufs=3) as kt_pool,
            tc.tile_pool(name="v", bufs=4) as v_pool,
            tc.tile_pool(name="p", bufs=3) as p_pool,
            tc.tile_pool(name="attn", bufs=3) as a_pool,
            tc.tile_pool(name="r", bufs=2) as r_pool,
            # PSUM allocation order fixes bank placement.
            tc.tile_pool(name="psA", bufs=1, space="PSUM") as psA_pool,
            tc.tile_pool(name="psB", bufs=1, space="PSUM") as psB_pool,
            tc.tile_pool(name="psZ", bufs=1, space="PSUM") as psZ_pool,
            tc.tile_pool(name="psO", bufs=1, space="PSUM") as psO_pool,
        ):
            qt_sb = singles.tile([128, BP, QL], BF)
            nc.sync.dma_start(out=qt_sb, in_=qH[:, :, :])
            id_sb = singles.tile([128, 128], BF)
            nc.sync.dma_start(out=id_sb, in_=ident[:, :])

            # Persistent PSUM accumulators, one full bank each: tile j
            # holds batch pairs 2j (cols 0-255) and 2j+1 (cols 256-511);
            # within a pair, even b on partitions 0-63, odd b on 64-127.
            outacc = [psO_pool.tile([128, 2 * QL], F32, tag=f"o{i}", name=f"outacc{i}")
                      for i in range(BP // 2)]

            Ps, As, Vs = {}, {}, {}

            def emit_mm1_exp(g):
                kt_sb = kt_pool.tile([128, BP, GK], BF, tag="kt", name=f"kt{g}")
                nc.sync.dma_start(out=kt_sb, in_=kH[g])
                v_sb = v_pool.tile([128, TK, B, D], BF, tag="v", name=f"v{g}")
                nc.sync.dma_start(out=v_sb, in_=vH[g])
                Vs[g] = v_sb
                P = p_pool.tile([128, B, TK * QL], BF, tag="p", name=f"P{g}")
                Ps[g] = P
                BS = TK * QL
                for t in range(TK):
                    for slot, bl in SLOT_SCHED:
                        nb = len(bl)
                        if slot == "A":
                            s_ps = psA_pool.tile([128, 4 * QL], F32, tag="sa",
                                                 name=f"sa{g}_{t}")
                        else:
                            s_ps = psB_pool.tile([128, 2 * QL], F32, tag="sb",
                                                 name=f"sb{g}_{t}")
                        for i, b in enumerate(bl):
                            bo, bp = b % 2, b // 2
                            # start=True lazy-zeroes the whole 2KB PSUM bank:
                            # exactly one start/stop pair per bank (two
                            # 256-col slices share each bank).
                            nc.tensor.matmul(
                                out=s_ps[:, i * QL:(i + 1) * QL],
                                lhsT=kt_sb[bo * 64:(bo + 1) * 64, bp,
                                           t * KT:(t + 1) * KT],
                                rhs=qt_sb[bo * 64:(bo + 1) * 64, bp, :],
                                start=(i % 2 == 0), stop=(i % 2 == 1),
                            )
                        # exp writes P[b] slices in PSUM stream order; the
                        # b-permutation becomes a strided 3/4D output AP.
                        p_ap = P[:, :, :]
                        off = p_ap.offset + bl[0] * BS + t * QL
                        if nb == 4:
                            free = [[(bl[2] - bl[0]) * BS, 2],
                                    [(bl[1] - bl[0]) * BS, 2], [1, QL]]
                            in_ap = s_ps[:, :].rearrange(
                                "p (o i q) -> p o i q", o=2, i=2)
                        else:
                            free = [[(bl[1] - bl[0]) * BS, 2], [1, QL]]
                            in_ap = s_ps[:, :].rearrange(
                                "p (i q) -> p i q", i=2)
                        out_ap = bass.AP(tensor=p_ap.tensor, offset=off,
                                         ap=[p_ap.ap[0]] + free)
                        nc.scalar.activation(
                            out=out_ap, in_=in_ap,
                            func=mybir.ActivationFunctionType.Exp,
                            scale=SCALE,
                        )

            def emit_norm(g):
                P = Ps[g]
                # Z = sum_b P[b] : identity-matmul accumulation in PSUM.
                Zp = psZ_pool.tile([128, TK * QL], F32, tag="z", name=f"z{g}")
                for b in range(B):
                    nc.tensor.matmul(
                        out=Zp[:, :], lhsT=id_sb[:, :], rhs=P[:, b, :],
                        start=(b == 0), stop=(b == B - 1),
                    )
                Rf = r_pool.tile([128, TK * QL], F32, tag="rf", name=f"rf{g}")
                Rs = r_pool.tile([128, TK * QL], F32, tag="rs", name=f"rs{g}")
                Rb = r_pool.tile([128, TK * QL], BF, tag="rb", name=f"rb{g}")
                nc.vector.reciprocal_approx_accurate(out=Rf[:, :], in_=Zp[:, :],
                                                     scratch=Rs[:, :])
                nc.vector.tensor_copy(out=Rb[:, :], in_=Rf[:, :])
                A = a_pool.tile([128, B, TK * QL], BF, tag="a", name=f"A{g}")
                As[g] = A
                rb_ap = Rb[:, :]
                rb_bcast = bass.AP(tensor=rb_ap.tensor, offset=rb_ap.offset,
                                   ap=[rb_ap.ap[0], [0, B], rb_ap.ap[1]])
                nc.vector.tensor_mul(out=A[:, :, :], in0=Ps[g][:, :, :],
                                     in1=rb_bcast)

            def emit_mm2(g, first, last):
                A, v_sb = As.pop(g), Vs.pop(g)
                Ps.pop(g, None)
                for b in range(B):
                    bo, bp = b % 2, b // 2
                    for t in range(TK):
                        nc.tensor.matmul(
                            out=outacc[bp // 2][bo * 64:(bo + 1) * 64,
                                                (bp % 2) * QL:(bp % 2 + 1) * QL],
                            lhsT=v_sb[:, t, b, :],
                            rhs=A[:, b, t * QL:(t + 1) * QL],
                            # zero regions are per-partition banks: the two
                            # col-slices (bp%2) share one; partition halves
                            # (bo) are independent.
                            start=(first and t == 0 and bp % 2 == 0),
                            stop=(last and t == TK - 1 and bp % 2 == 1),
                            # CoreSim's group tracker is partition-base
                            # blind; data semantics verified separately.
                            skip_group_check=True,
                        )

            import contextlib
            rep_ctx = tc.For_i(0, repeat, 1) if repeat > 1 else contextlib.nullcontext()
            with rep_ctx:
                for g in range(NG):
                    emit_mm1_exp(g)
                    if g >= 1:
                        emit_norm(g - 1)
                    if g >= 2:
                        emit_mm2(g - 2, first=(g == 2), last=False)
                emit_norm(NG - 1)
                emit_mm2(NG - 2, first=False, last=False)
                emit_mm2(NG - 1, first=False, last=True)

                out_sb = singles.tile([128, BP, QL], F32)
                for i in range(BP // 2):
                    nc.vector.tensor_copy(
                        out=out_sb[:, 2 * i:2 * i + 2, :],
                        in_=outacc[i][:, :].rearrange("p (j q) -> p j q", j=2),
                    )
                nc.sync.dma_start(out=outH[:, :, :], in_=out_sb)

    nc.finalize()   # Bacc.compile(): reg alloc + wait legalization
    return nc


_NC_CACHE = None


def _get_program():
    global _NC_CACHE
    if _NC_CACHE is None:
        _NC_CACHE = build_program()
    return _NC_CACHE


def make_in_maps(queries, keys, values):
    """Host-side staging into SBUF partition-images (bf16)."""
    # kH[g, 64*bo + d, bp, k'] = K[2*bp + bo, g*GK + k', d]
    kH = np.ascontiguousarray(
        keys.reshape(BP, 2, NG, GK, D).transpose(2, 1, 4, 0, 3)
    ).reshape(NG, 128, BP, GK).astype(bf16)
    # vH[g, p, t, b, d] = V[b, g*256 + t*128 + p, d]
    vH = np.ascontiguousarray(
        values.reshape(B, NG, TK, KT, D).transpose(1, 3, 2, 0, 4)
    ).astype(bf16)
    in_maps = []
    for c in range(NCORES):
        qs = queries[:, c * QL:(c + 1) * QL, :]          # [B, QL, D]
        qHc = np.ascontiguousarray(
            qs.reshape(BP, 2, QL, D).transpose(1, 3, 0, 2)
        ).reshape(128, BP, QL).astype(bf16)
        in_maps.append({"qH": qHc, "kH": kH, "vH": vH})
    return in_maps


def assemble_output(results):
    """outH [128, BP, QL] per core -> [B, N, D] full output."""
    out = np.empty((B, N, D), dtype=np.float32)
    for c, res in enumerate(results):
        oc = res["outH"].reshape(2, D, BP, QL).transpose(2, 0, 3, 1)  # [bp, bo, q, d]
        out[:, c * QL:(c + 1) * QL, :] = oc.reshape(B, QL, D)
    return out


def kernel(queries, keys, values):
    nc = _get_program()
    in_maps = make_in_maps(queries, keys, values)
    res = run_bass_kernel_spmd(nc, in_maps, core_ids=list(range(NCORES)))
    return assemble_output(res.results)


if __name__ == "__main__":
    rng = np.random.default_rng(0)
    q = rng.standard_normal((B, N, D), dtype=np.float32)
    k = rng.standard_normal((B, N, D), dtype=np.float32)
    v = rng.standard_normal((B, N, D), dtype=np.float32)
    o = kernel(queries=q, keys=k, values=v)
    print("kernel output", o.shape, o.dtype)


# revision 28
# speedup vs baseline: 70.2399x; 1.3444x over previous
"""Batch-softmax dot-product attention on 8 trn2 NeuronCores.

reference:  S = einsum('bqd,bkd->bqk', Q, K) / sqrt(D)
            A = softmax(S, axis=0)            # over the BATCH dim!
            out = einsum('bqk,bkd->bqd', A, V)

Sharding: split the QUERY dim across the 8 cores (256 queries each).
The softmax couples (q, k) positions across batches only, so with all
16 batches resident per core the kernel is embarrassingly parallel —
no collectives.

Host staging: every DRAM tensor is laid out as the exact SBUF
partition-image the kernel wants, so each DMA is one contiguous
multi-KB chunk per partition (descriptor count, not bandwidth, was the
bottleneck with natural layouts). Partition packing for Q/K/out:
p = 64*(b%2) + d — odd batches live on partitions 64-127, which both
the mm1 row tiling (T0/T8) and the mm2 column tiling need.

Per-core dataflow (all matmuls bf16 in / f32 PSUM out):
  mm1   S^T[b] (ktile 128 x q 256) = K^T[b] . Q[b]   contract d=64,
        row-tiled: even b uses PE tile T0, odd b T8. HW constraint
        (probed): T0/T8 must not alternate within one PSUM bank
        accumulation group, so the slot schedule fills each bank with
        uniform-parity batches.
  exp   ACT: P = exp(S^T / 8)  f32 PSUM -> bf16 SBUF (max-free
        softmax: scores are ~N(0,1), exp never overflows)
  zsum  Z = sum_b P[b] via PE identity-matmul accumulation (exact f32)
  recip R = 1/Z (DVE reciprocal_approx_accurate) -> bf16
  mul   attn = P * R (DVE, broadcast R over b)
  mm2   outT pair [128(2 x d64), q256] += V[b]^T . attn[b], accumulated
        over all 16 ktiles in PSUM; odd b targets PSUM partitions
        64-127 via PE column tiling.

Software pipeline (PE is in-order; without it the PE queue stalls on
the DVE recip+mul chain every group): iteration g emits
  mm1+exp(g) | zsum(g-1) recip/mul(g-1) | mm2(g-2)
so the PE always has mm1(g) ready while DVE normalizes g-1.

PSUM per partition (16KB): slotA 4KB + slotB 2KB (S^T staging) +
Z 2KB + 4 outT pair-accumulator banks 8KB.
"""

import numpy as np
import ml_dtypes

import concourse.bass as bass
import concourse.bacc as bacc
import concourse.tile as tile_mod
from concourse import mybir
from concourse.bass_utils import run_bass_kernel_spmd

B, N, D = 16, 2048, 64
NCORES = 8
QL = N // NCORES           # 256 queries per core
KT = 128                   # keys per kt tile
TK = 2                     # kt tiles per group
NG = N // (KT * TK)        # 8 groups
BP = B // 2                # 8 batch pairs
GK = TK * KT               # 256 keys per group
BF = mybir.dt.bfloat16
F32 = mybir.dt.float32
SCALE = 1.0 / np.sqrt(D)

bf16 = ml_dtypes.bfloat16

# mm1/exp slot schedule per kt tile: each PSUM bank is filled by a
# uniform-parity batch pair (HW row-tiling constraint).
SLOT_SCHED = [
    ("A", [0, 2, 1, 3]),
    ("B", [4, 6]),
    ("A", [5, 7, 8, 10]),
    ("B", [9, 11]),
    ("A", [12, 14, 13, 15]),
]


def build_program(repeat=1):
    # Bacc (not raw Bass): its compile() pass moves extra matmul waits onto
    # ldweights / event-semaphores, which walrus codegen requires.
    nc = bacc.Bacc(trn_type="TRN2")

    # SBUF partition-images (see module docstring).
    qH = nc.dram_tensor("qH", [128, BP, QL], BF, kind="ExternalInput")
    kH = nc.dram_tensor("kH", [NG, 128, BP, GK], BF, kind="ExternalInput")
    vH = nc.dram_tensor("vH", [NG, 128, TK, B, D], BF, kind="ExternalInput")
    outH = nc.dram_tensor("outH", [128, BP, QL], F32, kind="ExternalOutput")

    ident = nc.inline_tensor(np.eye(128, dtype=bf16), name="ident")

    with tile_mod.TileContext(nc) as tc:
        with (
            tc.tile_pool(name="singles", bufs=1) as singles,
            tc.tile_pool(name="kt", bufs=4) as kt_pool,
            tc.tile_pool(name="v", bufs=4) as v_pool,
            tc.tile_pool(name="p", bufs=4) as p_pool,
            tc.tile_pool(name="attn", bufs=3) as a_pool,
            tc.tile_pool(name="r", bufs=3) as r_pool,
            # PSUM allocation order fixes bank placement.
            tc.tile_pool(name="psA", bufs=1, space="PSUM") as psA_pool,
            tc.tile_pool(name="psB", bufs=1, space="PSUM") as psB_pool,
            tc.tile_pool(name="psZ", bufs=1, space="PSUM") as psZ_pool,
            tc.tile_pool(name="psO", bufs=1, space="PSUM") as psO_pool,
        ):
            qt_sb = singles.tile([128, BP, QL], BF)
            nc.sync.dma_start(out=qt_sb, in_=qH[:, :, :])
            id_sb = singles.tile([128, 128], BF)
            nc.sync.dma_start(out=id_sb, in_=ident[:, :])

            # Persistent PSUM accumulators, one full bank each: tile j
            # holds batch pairs 2j (cols 0-255) and 2j+1 (cols 256-511);
            # within a pair, even b on partitions 0-63, odd b on 64-127.
            outacc = [psO_pool.tile([128, 2 * QL], F32, tag=f"o{i}", name=f"outacc{i}")
                      for i in range(BP // 2)]

            Ps, As, Vs = {}, {}, {}

            def emit_mm1_exp(g):
                kt_sb = kt_pool.tile([128, BP, GK], BF, tag="kt", name=f"kt{g}")
                nc.sync.dma_start(out=kt_sb, in_=kH[g])
                v_sb = v_pool.tile([128, TK, B, D], BF, tag="v", name=f"v{g}")
                nc.sync.dma_start(out=v_sb, in_=vH[g])
                Vs[g] = v_sb
                P = p_pool.tile([128, B, TK * QL], BF, tag="p", name=f"P{g}")
                Ps[g] = P
                BS = TK * QL
                for t in range(TK):
                    for slot, bl in SLOT_SCHED:
                        nb = len(bl)
                        if slot == "A":
                            s_ps = psA_pool.tile([128, 4 * QL], F32, tag="sa",
                                                 name=f"sa{g}_{t}")
                        else:
                            s_ps = psB_pool.tile([128, 2 * QL], F32, tag="sb",
                                                 name=f"sb{g}_{t}")
                        for i, b in enumerate(bl):
                            bo, bp = b % 2, b // 2
                            # start=True lazy-zeroes the whole 2KB PSUM bank:
                            # exactly one start/stop pair per bank (two
                            # 256-col slices share each bank).
                            nc.tensor.matmul(
                                out=s_ps[:, i * QL:(i + 1) * QL],
                                lhsT=kt_sb[bo * 64:(bo + 1) * 64, bp,
                                           t * KT:(t + 1) * KT],
                                rhs=qt_sb[bo * 64:(bo + 1) * 64, bp, :],
                                start=(i % 2 == 0), stop=(i % 2 == 1),
                            )
                        # exp writes P[b] slices in PSUM stream order; the
                        # b-permutation becomes a strided 3/4D output AP.
                        p_ap = P[:, :, :]
                        off = p_ap.offset + bl[0] * BS + t * QL
                        if nb == 4:
                            free = [[(bl[2] - bl[0]) * BS, 2],
                                    [(bl[1] - bl[0]) * BS, 2], [1, QL]]
                            in_ap = s_ps[:, :].rearrange(
                                "p (o i q) -> p o i q", o=2, i=2)
                        else:
                            free = [[(bl[1] - bl[0]) * BS, 2], [1, QL]]
                            in_ap = s_ps[:, :].rearrange(
                                "p (i q) -> p i q", i=2)
                        out_ap = bass.AP(tensor=p_ap.tensor, offset=off,
                                         ap=[p_ap.ap[0]] + free)
                        nc.scalar.activation(
                            out=out_ap, in_=in_ap,
                            func=mybir.ActivationFunctionType.Exp,
                            scale=SCALE,
                        )

            def emit_norm(g):
                P = Ps[g]
                # Z = sum_b P[b] : identity-matmul accumulation in PSUM.
                Zp = psZ_pool.tile([128, TK * QL], F32, tag="z", name=f"z{g}")
                for b in range(B):
                    nc.tensor.matmul(
                        out=Zp[:, :], lhsT=id_sb[:, :], rhs=P[:, b, :],
                        start=(b == 0), stop=(b == B - 1),
                    )
                Rf = r_pool.tile([128, TK * QL], F32, tag="rf", name=f"rf{g}")
                Rs = r_pool.tile([128, TK * QL], F32, tag="rs", name=f"rs{g}")
                Rb = r_pool.tile([128, TK * QL], BF, tag="rb", name=f"rb{g}")
                nc.vector.reciprocal_approx_accurate(out=Rf[:, :], in_=Zp[:, :],
                                                     scratch=Rs[:, :])
                nc.vector.tensor_copy(out=Rb[:, :], in_=Rf[:, :])
                A = a_pool.tile([128, B, TK * QL], BF, tag="a", name=f"A{g}")
                As[g] = A
                rb_ap = Rb[:, :]
                rb_bcast = bass.AP(tensor=rb_ap.tensor, offset=rb_ap.offset,
                                   ap=[rb_ap.ap[0], [0, B], rb_ap.ap[1]])
                nc.vector.tensor_mul(out=A[:, :, :], in0=Ps[g][:, :, :],
                                     in1=rb_bcast)

            def emit_mm2(g, first, last):
                A, v_sb = As.pop(g), Vs.pop(g)
                Ps.pop(g, None)
                for b in range(B):
                    bo, bp = b % 2, b // 2
                    for t in range(TK):
                        nc.tensor.matmul(
                            out=outacc[bp // 2][bo * 64:(bo + 1) * 64,
                                                (bp % 2) * QL:(bp % 2 + 1) * QL],
                            lhsT=v_sb[:, t, b, :],
                            rhs=A[:, b, t * QL:(t + 1) * QL],
                            # zero regions are per-partition banks: the two
                            # col-slices (bp%2) share one; partition halves
                            # (bo) are independent.
                            start=(first and t == 0 and bp % 2 == 0),
                            stop=(last and t == TK - 1 and bp % 2 == 1),
                            # CoreSim's group tracker is partition-base
                            # blind; data semantics verified separately.
                            skip_group_check=True,
                        )

            import contextlib
            rep_ctx = tc.For_i(0, repeat, 1) if repeat > 1 else contextlib.nullcontext()
            with rep_ctx:
                for g in range(NG):
                    emit_mm1_exp(g)
                    if g >= 1:
                        emit_norm(g - 1)
                    if g >= 2:
                        emit_mm2(g - 2, first=(g == 2), last=False)
                emit_norm(NG - 1)
                emit_mm2(NG - 2, first=False, last=False)
                emit_mm2(NG - 1, first=False, last=True)

                out_sb = singles.tile([128, BP, QL], F32)
                for i in range(BP // 2):
                    nc.vector.tensor_copy(
                        out=out_sb[:, 2 * i:2 * i + 2, :],
                        in_=outacc[i][:, :].rearrange("p (j q) -> p j q", j=2),
                    )
                nc.sync.dma_start(out=outH[:, :, :], in_=out_sb)

    nc.finalize()   # Bacc.compile(): reg alloc + wait legalization
    return nc


_NC_CACHE = None


def _get_program():
    global _NC_CACHE
    if _NC_CACHE is None:
        _NC_CACHE = build_program()
    return _NC_CACHE


def make_in_maps(queries, keys, values):
    """Host-side staging into SBUF partition-images (bf16)."""
    # kH[g, 64*bo + d, bp, k'] = K[2*bp + bo, g*GK + k', d]
    kH = np.ascontiguousarray(
        keys.reshape(BP, 2, NG, GK, D).transpose(2, 1, 4, 0, 3)
    ).reshape(NG, 128, BP, GK).astype(bf16)
    # vH[g, p, t, b, d] = V[b, g*256 + t*128 + p, d]
    vH = np.ascontiguousarray(
        values.reshape(B, NG, TK, KT, D).transpose(1, 3, 2, 0, 4)
    ).astype(bf16)
    in_maps = []
    for c in range(NCORES):
        qs = queries[:, c * QL:(c + 1) * QL, :]          # [B, QL, D]
        qHc = np.ascontiguousarray(
            qs.reshape(BP, 2, QL, D).transpose(1, 3, 0, 2)
        ).reshape(128, BP, QL).astype(bf16)
        in_maps.append({"qH": qHc, "kH": kH, "vH": vH})
    return in_maps


def assemble_output(results):
    """outH [128, BP, QL] per core -> [B, N, D] full output."""
    out = np.empty((B, N, D), dtype=np.float32)
    for c, res in enumerate(results):
        oc = res["outH"].reshape(2, D, BP, QL).transpose(2, 0, 3, 1)  # [bp, bo, q, d]
        out[:, c * QL:(c + 1) * QL, :] = oc.reshape(B, QL, D)
    return out


def kernel(queries, keys, values):
    nc = _get_program()
    in_maps = make_in_maps(queries, keys, values)
    res = run_bass_kernel_spmd(nc, in_maps, core_ids=list(range(NCORES)))
    return assemble_output(res.results)


if __name__ == "__main__":
    rng = np.random.default_rng(0)
    q = rng.standard_normal((B, N, D), dtype=np.float32)
    k = rng.standard_normal((B, N, D), dtype=np.float32)
    v = rng.standard_normal((B, N, D), dtype=np.float32)
    o = kernel(queries=q, keys=k, values=v)
    print("kernel output", o.shape, o.dtype)
